# revision 47
# baseline (speedup 1.0000x reference)
"""Trainium2 Bass kernel for nn_BaselineGPT (sliding-window GQA attention block).

Sharding: 8 cores = 2 batches x 4 sequence chunks of 512 queries.
Each core computes its 512 output rows end-to-end (QKV proj, RMS norm, RoPE,
windowed GQA attention, output proj).  KV halo of 256 rows comes with the
chunk; chunk 0's missing halo is masked via a -30000 per-partition bias folded
into the exp() activation.  Pair-head mixing is folded into Wo on the host.

v2: software-pipelined schedule (PE never waits on the softmax chain),
sliding-window masks via gpsimd.affine_select, denominator handled by
DMA-broadcast + fast reciprocal, rope in bf16, O-proj interleaved with
attention.
"""

import math
from contextlib import ExitStack

import numpy as np

import concourse.bass as bass
from concourse import bacc
import concourse.mybir as mybir
import concourse.tile as tile
from concourse.masks import make_identity

B, S, DIM = 2, 2048, 1024
H, KVH, HD = 16, 4, 64
WINDOW = 256
ROPE_BASE = 10000.0
EPS = 1e-6

NQ = 512          # queries per core
NK = 768          # kv rows per core (incl 256 halo)
NCORES = 8
F32 = mybir.dt.float32
BF16 = mybir.dt.bfloat16

_BUILT = None


def _build():
    nc = bacc.Bacc(None)

    xt = nc.declare_dram_parameter("xt", [DIM, NK], BF16, isOutput=False)
    wq = nc.declare_dram_parameter("wq", [DIM, DIM], BF16, isOutput=False)
    wkv = nc.declare_dram_parameter("wkv", [DIM, 2 * KVH * HD], BF16, isOutput=False)
    wo = nc.declare_dram_parameter("wo", [DIM, DIM], BF16, isOutput=False)
    cosk = nc.declare_dram_parameter("cosk", [NK, HD // 2], BF16, isOutput=False)
    sink = nc.declare_dram_parameter("sink", [NK, HD // 2], BF16, isOutput=False)
    kb2 = nc.declare_dram_parameter("kb2", [128, 2], F32, isOutput=False)
    qgain = nc.declare_dram_parameter("qgain", [1, H], F32, isOutput=False)
    out = nc.declare_dram_parameter("out", [NQ, DIM], F32, isOutput=True)
    recs = nc.dram_tensor("recs", [16, 512], F32)

    with tile.TileContext(nc) as tc, ExitStack() as ctx:
        const = ctx.enter_context(tc.tile_pool(name="const", bufs=1))
        big = ctx.enter_context(tc.tile_pool(name="big", bufs=1))
        tmp = ctx.enter_context(tc.tile_pool(name="tmp", bufs=3))
        att_pool = ctx.enter_context(tc.tile_pool(name="att", bufs=3))
        yr_pool = ctx.enter_context(tc.tile_pool(name="yr", bufs=6))
        rb_pool = ctx.enter_context(tc.tile_pool(name="rb", bufs=3))
        tn_pool = ctx.enter_context(tc.tile_pool(name="tn", bufs=3))
        ob_pool = ctx.enter_context(tc.tile_pool(name="ob", bufs=2))
        pb = ctx.enter_context(tc.tile_pool(name="pb", bufs=6, space="PSUM"))
        pbt = ctx.enter_context(tc.tile_pool(name="pbt", bufs=2, space="PSUM"))

        # ---- constants / small inputs (vector/gpsimd; tiny) ----
        ident = const.tile([128, 128], BF16, tag="ident")
        make_identity(nc, ident)
        eps_t = const.tile([128, 1], F32, tag="eps")
        nc.vector.memset(eps_t, EPS)
        ones1x64 = const.tile([1, 64], BF16, tag="ones1x64")
        nc.vector.memset(ones1x64, 1.0)
        # consts go on the scalar queue so the sync queue starts on xt
        # immediately; cos/sin land as two strided DMAs
        qg_sb = const.tile([128, H], F32, tag="qg")
        nc.scalar.dma_start(out=qg_sb, in_=qgain[0:1, :].to_broadcast((128, H)))
        kb_sb = const.tile([128, 2], F32, tag="kb")
        nc.scalar.dma_start(out=kb_sb, in_=kb2[:, :])
        cos_all = const.tile([128, 6, HD // 2], BF16, tag="cos")
        nc.scalar.dma_start(
            out=cos_all, in_=cosk.rearrange("(t p) f -> p t f", p=128)
        )
        sin_all = const.tile([128, 6, HD // 2], BF16, tag="sin")
        nc.scalar.dma_start(
            out=sin_all, in_=sink.rearrange("(t p) f -> p t f", p=128)
        )
        cos_sb = [cos_all[:, st, :] for st in range(6)]
        sin_sb = [sin_all[:, st, :] for st in range(6)]

        # ---- big persistent SBUF tensors; one bulk DMA per tensor, spread
        # over 4 queues (k-tile kt lives at dim1 index kt) ----
        xt_all = big.tile([128, 8, NK], BF16, tag="xt", name="xt_all")
        xt_r = xt.rearrange("(k p) c -> p k c", p=128)
        for k in range(8):
            nc.sync.dma_start(out=xt_all[:, k, :], in_=xt_r[:, k, :])
        wkv_all = big.tile([128, 8, 512], BF16, tag="wkv", name="wkv_all")
        wkv_r = wkv.rearrange("(k p) c -> p k c", p=128)
        for k in range(2):
            nc.gpsimd.dma_start(
                out=wkv_all[:, 4 * k : 4 * k + 4, :], in_=wkv_r[:, 4 * k : 4 * k + 4, :]
            )
        wq_all = big.tile([128, 8, DIM], BF16, tag="wq", name="wq_all")
        wq_r = wq.rearrange("(k p) c -> p k c", p=128)
        for k in range(4):
            (nc.scalar if k < 2 else nc.sync).dma_start(
                out=wq_all[:, 2 * k : 2 * k + 2, :], in_=wq_r[:, 2 * k : 2 * k + 2, :]
            )
        wo_all = big.tile([128, 8, DIM], BF16, tag="wo", name="wo_all")
        wo_r = wo.rearrange("(k p) c -> p k c", p=128)
        for k in range(4):
            nc.gpsimd.dma_start(
                out=wo_all[:, 2 * k : 2 * k + 2, :], in_=wo_r[:, 2 * k : 2 * k + 2, :]
            )
        xt_sb = [xt_all[:, k, :] for k in range(8)]
        wkv_sb = [wkv_all[:, k, :] for k in range(8)]
        wq_sb = [wq_all[:, k, :] for k in range(8)]
        wo_sb = [wo_all[:, k, :] for k in range(8)]

        q_rope = big.tile([128, 4, DIM], BF16, tag="qrope")
        k_rope = big.tile([128, 6, KVH * HD], BF16, tag="krope")
        v_sb = big.tile([128, 6, KVH, HD + 1], BF16, tag="v")
        kt2_sb = big.tile([128, 2, NK], BF16, tag="kt2")
        qt2_sb = big.tile([128, 8, 512], BF16, tag="qt2")
        yt_sb = big.tile([128, 8, NQ], BF16, tag="yt")
        nc.vector.memset(v_sb[:, :, :, HD : HD + 1], 1.0)

        def norm_stats(src_psum, nheads, ssq, col):
            """square+reduce of src_psum [128, nheads*HD] into ssq[:, col:]."""
            src = src_psum.rearrange("p (h d) -> p h d", d=HD)
            sq = tmp.tile([128, 16, HD], F32, tag="sq")
            nc.scalar.activation(
                out=sq[:, :nheads, :], in_=src,
                func=mybir.ActivationFunctionType.Square,
            )
            nc.vector.tensor_reduce(
                out=ssq[:, col : col + nheads], in_=sq[:, :nheads, :],
                axis=mybir.AxisListType.X, op=mybir.AluOpType.add,
            )

        def norm_finish(ssq, ncols, gain):
            """ssq -> inv = 1/sqrt(ssq/HD + eps) (batched), optional gain."""
            nc.scalar.activation(
                out=ssq[:, :ncols], in_=ssq[:, :ncols],
                func=mybir.ActivationFunctionType.Sqrt,
                bias=eps_t, scale=1.0 / HD,
            )
            inv = tmp.tile([128, 16], F32, tag="inv")
            nc.vector.reciprocal_approx_fast(out=inv[:, :ncols], in_=ssq[:, :ncols])
            if gain:
                nc.vector.tensor_mul(
                    out=inv[:, :ncols], in0=inv[:, :ncols], in1=qg_sb[:, :ncols]
                )
            return inv

        def rope_apply(src_psum, nheads, st, dst, inv, icol):
            """normalize src by inv[:, icol:] then RoPE at kv tile st -> dst."""
            src = src_psum.rearrange("p (h d) -> p h d", d=HD)
            invf = tmp.tile([128, 16, HD], F32, tag="invf")
            nc.vector.tensor_copy(
                out=invf[:, :nheads, :],
                in_=inv[:, icol : icol + nheads]
                .rearrange("p (h o) -> p h o", o=1)
                .broadcast_to((128, nheads, HD)),
            )
            rn = tmp.tile([128, 16, HD], BF16, tag="rn")
            nc.vector.tensor_mul(
                out=rn[:, :nheads, :], in0=src, in1=invf[:, :nheads, :]
            )
            # RoPE in bf16: out1 = r1*cos + r2*sin ; out2 = r2*cos - r1*sin
            hd2 = HD // 2
            r1 = rn[:, :nheads, 0:hd2]
            r2 = rn[:, :nheads, hd2:HD]
            cosb = cos_sb[st].rearrange("p (o f) -> p o f", o=1).broadcast_to(
                (128, nheads, hd2)
            )
            sinb = sin_sb[st].rearrange("p (o f) -> p o f", o=1).broadcast_to(
                (128, nheads, hd2)
            )
            dd = dst.rearrange("p (h d) -> p h d", d=HD)
            o1 = dd[:, :, 0:hd2]
            o2 = dd[:, :, hd2:HD]
            # o1 half on the vector engine, o2 half on the pool engine
            # (rn/cos/sin/dst are all SBUF, which pool can reach)
            t1 = tmp.tile([128, 16, hd2], BF16, tag="ropet1")
            t2 = tmp.tile([128, 16, hd2], BF16, tag="ropet2")
            nc.vector.tensor_mul(out=t1[:, :nheads, :], in0=r1, in1=cosb)
            nc.vector.tensor_mul(out=t2[:, :nheads, :], in0=r2, in1=sinb)
            nc.vector.tensor_add(
                out=o1, in0=t1[:, :nheads, :], in1=t2[:, :nheads, :]
            )
            eng = nc.gpsimd if nheads == 8 else nc.vector
            t3 = tmp.tile([128, 16, hd2], BF16, tag="ropet3")
            t4 = tmp.tile([128, 16, hd2], BF16, tag="ropet4")
            eng.tensor_mul(out=t3[:, :nheads, :], in0=r2, in1=cosb)
            eng.tensor_mul(out=t4[:, :nheads, :], in0=r1, in1=sinb)
            eng.tensor_sub(
                out=o2, in0=t3[:, :nheads, :], in1=t4[:, :nheads, :]
            )

        # ---- fused K|V projection over 6 kv s-tiles, st pairs share one
        # batched rsqrt ----
        for sp in range(3):
            pkvs = []
            ssq = tmp.tile([128, 16], F32, tag="ssq")
            for j in range(2):
                st = 2 * sp + j
                pkv = pb.tile([128, 512], F32, tag="pb")
                for kt_ in range(8):
                    nc.tensor.matmul(
                        out=pkv,
                        lhsT=xt_sb[kt_][:, st * 128 : st * 128 + 128],
                        rhs=wkv_sb[kt_],
                        start=(kt_ == 0),
                        stop=(kt_ == 7),
                    )
                nc.scalar.activation(
                    out=v_sb[:, st, :, 0:HD],
                    in_=pkv[:, KVH * HD :].rearrange("p (g d) -> p g d", d=HD),
                    func=mybir.ActivationFunctionType.Copy,
                )
                norm_stats(pkv[:, 0 : KVH * HD], KVH, ssq, j * KVH)
                pkvs.append(pkv)
            inv = norm_finish(ssq, 2 * KVH, gain=False)
            for j in range(2):
                st = 2 * sp + j
                rope_apply(
                    pkvs[j][:, 0 : KVH * HD], KVH, st, k_rope[:, st, :],
                    inv, j * KVH,
                )

        # ---- Q projection over 4 q s-tiles (kv rows 256..768), halves
        # share one batched rsqrt ----
        for st in range(4):
            pqs = []
            ssq = tmp.tile([128, 16], F32, tag="ssq")
            for half in range(2):
                pq = pb.tile([128, 512], F32, tag="pb")
                for kt_ in range(8):
                    nc.tensor.matmul(
                        out=pq,
                        lhsT=xt_sb[kt_][:, 256 + st * 128 : 384 + st * 128],
                        rhs=wq_sb[kt_][:, half * 512 : half * 512 + 512],
                        start=(kt_ == 0),
                        stop=(kt_ == 7),
                    )
                norm_stats(pq, 8, ssq, half * 8)
                pqs.append(pq)
            inv = norm_finish(ssq, 16, gain=True)
            for half in range(2):
                rope_apply(
                    pqs[half], 8, st + 2,
                    q_rope[:, st, half * 512 : half * 512 + 512],
                    inv, half * 8,
                )

        # ---- transpose K: k_rope [128s, (g,d)] -> kt2_sb [2*64d, gpair, s] ----
        for st in range(6):
            ptk = pbt.tile([128, 512], BF16, tag="pbt")
            for gp in range(2):
                nc.tensor.transpose(
                    out=ptk[:, gp * 128 : gp * 128 + 128],
                    in_=k_rope[:, st, gp * 128 : gp * 128 + 128],
                    identity=ident,
                )
            nc.vector.tensor_copy(
                out=kt2_sb[:, :, st * 128 : st * 128 + 128],
                in_=ptk[:, 0:256].rearrange("p (g s) -> p g s", s=128),
            )

        # ---- transpose Q (just-in-time, interleaved with attention):
        # q_rope -> qt2_sb[:, gp*4+qb, :] (2 groups stacked) ----
        def emit_qtr(gp, qb):
            ptq = pbt.tile([128, 512], BF16, tag="pbt")
            for gl in range(2):
                g = gp * 2 + gl
                for hh in range(4):
                    h = g * 4 + hh
                    nc.tensor.transpose(
                        out=ptq[gl * 64 : gl * 64 + 64, hh * 128 : hh * 128 + 128],
                        in_=q_rope[:, qb, h * HD : h * HD + HD],
                        identity=ident,
                    )
            if (gp * 4 + qb) % 2 == 0:
                nc.scalar.activation(
                    out=qt2_sb[:, gp * 4 + qb, :], in_=ptq,
                    func=mybir.ActivationFunctionType.Copy,
                )
            else:
                nc.vector.tensor_copy(out=qt2_sb[:, gp * 4 + qb, :], in_=ptq)

        # ---- attention + O-proj, software-pipelined over it = qb*4 + g ----
        Exp = mybir.ActivationFunctionType.Exp

        def emit_scores(it):
            qb, g = it // 4, it % 4
            gp, gl = g // 2, g % 2
            att = att_pool.tile([128, 3, 512], BF16, tag="att")
            for t in range(3):
                pss = pb.tile([128, 512], F32, tag="pb")
                nc.tensor.matmul(
                    out=pss,
                    lhsT=kt2_sb[
                        gl * 64 : gl * 64 + 64, gp,
                        qb * 128 + t * 128 : qb * 128 + t * 128 + 128,
                    ],
                    rhs=qt2_sb[gl * 64 : gl * 64 + 64, gp * 4 + qb, :],
                    start=True,
                    stop=True,
                )
                kt_idx = qb + t
                if kt_idx < 2:
                    nc.scalar.activation(
                        out=att[:, t, :], in_=pss, func=Exp,
                        bias=kb_sb[:, kt_idx : kt_idx + 1],
                    )
                else:
                    nc.scalar.activation(out=att[:, t, :], in_=pss, func=Exp)
            # sliding-window masks: tile0 keep k>q, tile2 keep q>=k
            nc.gpsimd.affine_select(
                out=att[:, 0, :], in_=att[:, 0, :],
                compare_op=mybir.AluOpType.is_ge, fill=0.0,
                base=-1, channel_multiplier=1, pattern=[[0, 4], [-1, 128]],
            )
            nc.gpsimd.affine_select(
                out=att[:, 2, :], in_=att[:, 2, :],
                compare_op=mybir.AluOpType.is_ge, fill=0.0,
                base=0, channel_multiplier=-1, pattern=[[0, 4], [1, 128]],
            )
            return att

        def emit_av(it, att):
            qb, g = it // 4, it % 4
            psy = pb.tile([128, 512], F32, tag="pb")
            for t in range(3):
                nc.tensor.matmul(
                    out=psy[0:65, :],
                    lhsT=v_sb[:, qb + t, g, :],
                    rhs=att[:, t, :],
                    start=(t == 0),
                    stop=(t == 2),
                )
            # y rows to SBUF (frees the bank); den row staged at partition 0
            # (reciprocal_approx_fast needs a base-0 SBUF input on HW)
            yr = yr_pool.tile([64, 512], F32, tag="yr")
            if it % 2 == 0:
                nc.vector.tensor_copy(out=yr, in_=psy[0:64, :])
            else:
                nc.scalar.activation(
                    out=yr, in_=psy[0:64, :],
                    func=mybir.ActivationFunctionType.Copy,
                )
            dr = tn_pool.tile([1, 512], F32, tag="dr")
            nc.scalar.activation(
                out=dr, in_=psy[64:65, :],
                func=mybir.ActivationFunctionType.Copy,
            )
            rec = rb_pool.tile([1, 512], F32, tag="rec")
            nc.vector.reciprocal_approx_fast(out=rec, in_=dr)
            rec16 = rb_pool.tile([1, 512], BF16, tag="rec16")
            nc.vector.tensor_copy(out=rec16, in_=rec)
            # broadcast 1/den across 64 partitions with a ones-column matmul
            rb_ps = pb.tile([128, 512], F32, tag="pb")
            nc.tensor.matmul(
                out=rb_ps[0:64, :], lhsT=ones1x64, rhs=rec16,
                start=True, stop=True,
            )
            tn = tn_pool.tile([64, 512], BF16, tag="tn")
            nc.vector.tensor_mul(out=tn, in0=yr, in1=rb_ps[0:64, :])
            # scatter into yt pair layout: hh 0,2 -> partitions 0:64 of
            # pairs (2g, 2g+1); hh 1,3 -> partitions 64:128
            tn4 = tn.rearrange("p (a q) -> p a q", q=128)
            for lohalf in range(2):
                (nc.sync if lohalf == 0 else nc.gpsimd).dma_start(
                    out=yt_sb[
                        lohalf * 64 : lohalf * 64 + 64,
                        2 * g : 2 * g + 2,
                        qb * 128 : qb * 128 + 128,
                    ],
                    in_=tn4[:, lohalf::2, :],
                )

        def emit_oproj(qb):
            ob = ob_pool.tile([128, DIM], F32, tag="ob")
            for half in range(2):
                po = pb.tile([128, 512], F32, tag="pb")
                for p in range(8):
                    nc.tensor.matmul(
                        out=po,
                        lhsT=yt_sb[:, p, qb * 128 : qb * 128 + 128],
                        rhs=wo_sb[p][:, half * 512 : half * 512 + 512],
                        start=(p == 0),
                        stop=(p == 7),
                    )
                if half == 0:
                    nc.scalar.activation(
                        out=ob[:, 0:512], in_=po,
                        func=mybir.ActivationFunctionType.Copy,
                    )
                else:
                    nc.vector.tensor_copy(out=ob[:, 512:1024], in_=po)
                nc.sync.dma_start(
                    out=out[
                        qb * 128 : qb * 128 + 128, half * 512 : half * 512 + 512
                    ],
                    in_=ob[:, half * 512 : half * 512 + 512],
                )

        atts = {}
        emit_qtr(0, 0)
        emit_qtr(1, 0)
        for it in range(16):
            qb, g = it // 4, it % 4
            if g == 0 and qb < 3:
                emit_qtr(0, qb + 1)
            atts[it] = emit_scores(it)
            if g == 2 and qb < 3:
                emit_qtr(1, qb + 1)
            if it >= 1:
                emit_av(it - 1, atts.pop(it - 1))
            if it in (6, 10, 14):
                emit_oproj((it - 6) // 4)
        emit_av(15, atts.pop(15))
        emit_oproj(3)

    nc.finalize()
    return nc


def _host_inputs(x, Wq, Wk, Wv, Wo, q_gain, pair_mix):
    """Build the 8 per-core input maps."""
    x = np.asarray(x, np.float32)
    Wq = np.asarray(Wq, np.float32)
    Wk = np.asarray(Wk, np.float32)
    Wv = np.asarray(Wv, np.float32)
    Wo = np.asarray(Wo, np.float32)
    q_gain = np.asarray(q_gain, np.float32)
    pair_mix = np.asarray(pair_mix, np.float32)

    # fold pair mixing into Wo:  out = y_mix @ Wo.T,  y_mix = y @ M.T  =>  Wo' = Wo @ M
    M = np.zeros((DIM, DIM), np.float32)
    eye = np.eye(HD, dtype=np.float32)
    for p in range(H // 2):
        for o in range(2):
            for i in range(2):
                ho, hi = 2 * p + o, 2 * p + i
                M[ho * HD : ho * HD + HD, hi * HD : hi * HD + HD] = (
                    pair_mix[p, o, i] * eye
                )
    woT = np.ascontiguousarray((Wo @ M).T)

    wqT = np.ascontiguousarray(Wq.T)
    wkvT = np.ascontiguousarray(
        np.concatenate([Wk.T, Wv.T], axis=1)
    )  # [DIM, 512]
    qg8 = (q_gain / math.sqrt(HD)).reshape(1, H).astype(np.float32)

    inv_freq = 1.0 / (ROPE_BASE ** (np.arange(0, HD, 2, dtype=np.float32) / HD))

    import ml_dtypes
    bf = ml_dtypes.bfloat16
    wqT, wkvT, woT = (a.astype(bf) for a in (wqT, wkvT, woT))
    in_maps = []
    for core in range(NCORES):
        b, c = core // 4, core % 4
        ks = 512 * c - 256
        xc = np.zeros((NK, DIM), np.float32)
        lo = max(0, ks)
        xc[lo - ks :] = x[b, lo : ks + NK]
        t = (ks + np.arange(NK, dtype=np.float32))[:, None]
        freqs = t * inv_freq[None, :]
        # exp-bias columns for k-tiles 0 and 1 (the halo region): -30000
        # where the global position is < 0 (chunk 0 only), else 0
        kbcols = np.zeros((128, 2), np.float32)
        for tt in range(2):
            gpos = ks + tt * 128 + np.arange(128)
            kbcols[:, tt] = np.where(gpos < 0, -30000.0, 0.0)
        in_maps.append(
            {
                "xt": np.ascontiguousarray(xc.T).astype(bf),
                "wq": wqT,
                "wkv": wkvT,
                "wo": woT,
                "cosk": np.cos(freqs).astype(bf),
                "sink": np.sin(freqs).astype(bf),
                "kb2": kbcols,
                "qgain": qg8,
            }
        )
    return in_maps


def kernel(x, Wq, Wk, Wv, Wo, q_gain, pair_mix):
    global _BUILT
    from concourse.bass_utils import run_bass_kernel_spmd

    if _BUILT is None:
        _BUILT = _build()
    in_maps = _host_inputs(x, Wq, Wk, Wv, Wo, q_gain, pair_mix)
    res = run_bass_kernel_spmd(_BUILT, in_maps, list(range(NCORES)))
    out = np.empty((B, S, DIM), np.float32)
    for core in range(NCORES):
        b, c = core // 4, core % 4
        out[b, 512 * c : 512 * c + 512, :] = res.results[core]["out"]
    return out


# revision 48
# speedup vs baseline: 1.2711x; 1.2711x over previous
"""Trainium2 Bass kernel for nn_BaselineGPT (sliding-window GQA attention block).

Sharding: 8 cores = 2 batches x 4 sequence chunks of 512 queries.
Each core computes its 512 output rows end-to-end (QKV proj, RMS norm, RoPE,
windowed GQA attention, output proj).  KV halo of 256 rows comes with the
chunk; chunk 0's missing halo is masked via a -30000 per-partition bias folded
into the exp() activation.  Pair-head mixing is folded into Wo on the host.

v2: software-pipelined schedule (PE never waits on the softmax chain),
sliding-window masks via gpsimd.affine_select, denominator handled by
DMA-broadcast + fast reciprocal, rope in bf16, O-proj interleaved with
attention.
"""

import math
from contextlib import ExitStack

import numpy as np

import concourse.bass as bass
from concourse import bacc
import concourse.mybir as mybir
import concourse.tile as tile
from concourse.masks import make_identity

B, S, DIM = 2, 2048, 1024
H, KVH, HD = 16, 4, 64
WINDOW = 256
ROPE_BASE = 10000.0
EPS = 1e-6

NQ = 512          # queries per core
NK = 768          # kv rows per core (incl 256 halo)
NCORES = 8
F32 = mybir.dt.float32
BF16 = mybir.dt.bfloat16

_BUILT = None


def _build():
    nc = bacc.Bacc(None)

    xt = nc.declare_dram_parameter("xt", [DIM, NK], BF16, isOutput=False)
    wq = nc.declare_dram_parameter("wq", [DIM, DIM], BF16, isOutput=False)
    wkv = nc.declare_dram_parameter("wkv", [DIM, 2 * KVH * HD], BF16, isOutput=False)
    wo = nc.declare_dram_parameter("wo", [DIM, DIM], BF16, isOutput=False)
    cosk = nc.declare_dram_parameter("cosk", [NK, HD // 2], BF16, isOutput=False)
    sink = nc.declare_dram_parameter("sink", [NK, HD // 2], BF16, isOutput=False)
    kb2 = nc.declare_dram_parameter("kb2", [128, 2], F32, isOutput=False)
    qgain = nc.declare_dram_parameter("qgain", [1, H], F32, isOutput=False)
    out = nc.declare_dram_parameter("out", [NQ, DIM], F32, isOutput=True)
    recs = nc.dram_tensor("recs", [16, 512], F32)

    with tile.TileContext(nc) as tc, ExitStack() as ctx:
        const = ctx.enter_context(tc.tile_pool(name="const", bufs=1))
        big = ctx.enter_context(tc.tile_pool(name="big", bufs=1))
        tmp = ctx.enter_context(tc.tile_pool(name="tmp", bufs=3))
        att_pool = ctx.enter_context(tc.tile_pool(name="att", bufs=3))
        yr_pool = ctx.enter_context(tc.tile_pool(name="yr", bufs=6))
        rb_pool = ctx.enter_context(tc.tile_pool(name="rb", bufs=3))
        tn_pool = ctx.enter_context(tc.tile_pool(name="tn", bufs=3))
        ob_pool = ctx.enter_context(tc.tile_pool(name="ob", bufs=2))
        pb = ctx.enter_context(tc.tile_pool(name="pb", bufs=6, space="PSUM"))
        pbt = ctx.enter_context(tc.tile_pool(name="pbt", bufs=2, space="PSUM"))

        # ---- constants / small inputs (vector/gpsimd; tiny) ----
        ident = const.tile([128, 128], BF16, tag="ident")
        make_identity(nc, ident)
        eps_t = const.tile([128, 1], F32, tag="eps")
        nc.vector.memset(eps_t, EPS)
        ones64 = const.tile([128, HD], BF16, tag="ones64")
        nc.vector.memset(ones64, 1.0)
        # consts go on the scalar queue so the sync queue starts on xt
        # immediately; cos/sin land as two strided DMAs
        qg_sb = const.tile([128, H], F32, tag="qg")
        nc.scalar.dma_start(out=qg_sb, in_=qgain[0:1, :].to_broadcast((128, H)))
        kb_sb = const.tile([128, 2], F32, tag="kb")
        nc.scalar.dma_start(out=kb_sb, in_=kb2[:, :])
        cos_all = const.tile([128, 6, HD // 2], BF16, tag="cos")
        nc.scalar.dma_start(
            out=cos_all, in_=cosk.rearrange("(t p) f -> p t f", p=128)
        )
        sin_all = const.tile([128, 6, HD // 2], BF16, tag="sin")
        nc.scalar.dma_start(
            out=sin_all, in_=sink.rearrange("(t p) f -> p t f", p=128)
        )
        cos_sb = [cos_all[:, st, :] for st in range(6)]
        sin_sb = [sin_all[:, st, :] for st in range(6)]

        # ---- big persistent SBUF tensors; one bulk DMA per tensor, spread
        # over 4 queues (k-tile kt lives at dim1 index kt) ----
        xt_all = big.tile([128, 8, NK], BF16, tag="xt", name="xt_all")
        xt_r = xt.rearrange("(k p) c -> p k c", p=128)
        for k in range(8):
            nc.sync.dma_start(out=xt_all[:, k, :], in_=xt_r[:, k, :])
        wkv_all = big.tile([128, 8, 512], BF16, tag="wkv", name="wkv_all")
        wkv_r = wkv.rearrange("(k p) c -> p k c", p=128)
        for k in range(2):
            nc.gpsimd.dma_start(
                out=wkv_all[:, 4 * k : 4 * k + 4, :], in_=wkv_r[:, 4 * k : 4 * k + 4, :]
            )
        wq_all = big.tile([128, 8, DIM], BF16, tag="wq", name="wq_all")
        wq_r = wq.rearrange("(k p) c -> p k c", p=128)
        for k in range(4):
            (nc.scalar if k < 2 else nc.sync).dma_start(
                out=wq_all[:, 2 * k : 2 * k + 2, :], in_=wq_r[:, 2 * k : 2 * k + 2, :]
            )
        wo_all = big.tile([128, 8, DIM], BF16, tag="wo", name="wo_all")
        wo_r = wo.rearrange("(k p) c -> p k c", p=128)
        for k in range(4):
            nc.gpsimd.dma_start(
                out=wo_all[:, 2 * k : 2 * k + 2, :], in_=wo_r[:, 2 * k : 2 * k + 2, :]
            )
        xt_sb = [xt_all[:, k, :] for k in range(8)]
        wkv_sb = [wkv_all[:, k, :] for k in range(8)]
        wq_sb = [wq_all[:, k, :] for k in range(8)]
        wo_sb = [wo_all[:, k, :] for k in range(8)]

        q_rope = big.tile([128, 4, DIM], BF16, tag="qrope")
        k_rope = big.tile([128, 6, KVH * HD], BF16, tag="krope")
        v_sb = big.tile([128, 6, KVH, HD + 1], BF16, tag="v")
        kt2_sb = big.tile([128, 2, NK], BF16, tag="kt2")
        qt2_sb = big.tile([128, 8, 512], BF16, tag="qt2")
        yt_sb = big.tile([128, 8, NQ], BF16, tag="yt")
        nc.vector.memset(v_sb[:, :, :, HD : HD + 1], 1.0)

        def norm_stats(src_psum, nheads, ssq, col):
            """square+reduce of src_psum [128, nheads*HD] into ssq[:, col:]."""
            src = src_psum.rearrange("p (h d) -> p h d", d=HD)
            sq = tmp.tile([128, 16, HD], F32, tag="sq")
            nc.scalar.activation(
                out=sq[:, :nheads, :], in_=src,
                func=mybir.ActivationFunctionType.Square,
            )
            nc.vector.tensor_reduce(
                out=ssq[:, col : col + nheads], in_=sq[:, :nheads, :],
                axis=mybir.AxisListType.X, op=mybir.AluOpType.add,
            )

        def norm_finish(ssq, ncols, gain):
            """ssq -> inv = 1/sqrt(ssq/HD + eps) (batched), optional gain."""
            nc.scalar.activation(
                out=ssq[:, :ncols], in_=ssq[:, :ncols],
                func=mybir.ActivationFunctionType.Sqrt,
                bias=eps_t, scale=1.0 / HD,
            )
            inv = tmp.tile([128, 16], F32, tag="inv")
            nc.vector.reciprocal_approx_fast(out=inv[:, :ncols], in_=ssq[:, :ncols])
            if gain:
                nc.vector.tensor_mul(
                    out=inv[:, :ncols], in0=inv[:, :ncols], in1=qg_sb[:, :ncols]
                )
            return inv

        def rope_apply(src_psum, nheads, st, dst, inv, icol):
            """normalize src by inv[:, icol:] then RoPE at kv tile st -> dst."""
            src = src_psum.rearrange("p (h d) -> p h d", d=HD)
            invf = tmp.tile([128, 16, HD], F32, tag="invf")
            nc.vector.tensor_copy(
                out=invf[:, :nheads, :],
                in_=inv[:, icol : icol + nheads]
                .rearrange("p (h o) -> p h o", o=1)
                .broadcast_to((128, nheads, HD)),
            )
            rn = tmp.tile([128, 16, HD], BF16, tag="rn")
            nc.vector.tensor_mul(
                out=rn[:, :nheads, :], in0=src, in1=invf[:, :nheads, :]
            )
            # RoPE in bf16: out1 = r1*cos + r2*sin ; out2 = r2*cos - r1*sin
            hd2 = HD // 2
            r1 = rn[:, :nheads, 0:hd2]
            r2 = rn[:, :nheads, hd2:HD]
            cosb = cos_sb[st].rearrange("p (o f) -> p o f", o=1).broadcast_to(
                (128, nheads, hd2)
            )
            sinb = sin_sb[st].rearrange("p (o f) -> p o f", o=1).broadcast_to(
                (128, nheads, hd2)
            )
            dd = dst.rearrange("p (h d) -> p h d", d=HD)
            o1 = dd[:, :, 0:hd2]
            o2 = dd[:, :, hd2:HD]
            # o1 half on the vector engine, o2 half on the pool engine
            # (rn/cos/sin/dst are all SBUF, which pool can reach)
            t1 = tmp.tile([128, 16, hd2], BF16, tag="ropet1")
            t2 = tmp.tile([128, 16, hd2], BF16, tag="ropet2")
            nc.vector.tensor_mul(out=t1[:, :nheads, :], in0=r1, in1=cosb)
            nc.vector.tensor_mul(out=t2[:, :nheads, :], in0=r2, in1=sinb)
            nc.vector.tensor_add(
                out=o1, in0=t1[:, :nheads, :], in1=t2[:, :nheads, :]
            )
            eng = nc.gpsimd if nheads == 8 else nc.vector
            t3 = tmp.tile([128, 16, hd2], BF16, tag="ropet3")
            t4 = tmp.tile([128, 16, hd2], BF16, tag="ropet4")
            eng.tensor_mul(out=t3[:, :nheads, :], in0=r2, in1=cosb)
            eng.tensor_mul(out=t4[:, :nheads, :], in0=r1, in1=sinb)
            eng.tensor_sub(
                out=o2, in0=t3[:, :nheads, :], in1=t4[:, :nheads, :]
            )

        # ---- fused K|V projection over 6 kv s-tiles, st pairs share one
        # batched rsqrt ----
        for sp in range(3):
            pkvs = []
            ssq = tmp.tile([128, 16], F32, tag="ssq")
            for j in range(2):
                st = 2 * sp + j
                pkv = pb.tile([128, 512], F32, tag="pb")
                for kt_ in range(8):
                    nc.tensor.matmul(
                        out=pkv,
                        lhsT=xt_sb[kt_][:, st * 128 : st * 128 + 128],
                        rhs=wkv_sb[kt_],
                        start=(kt_ == 0),
                        stop=(kt_ == 7),
                    )
                nc.scalar.activation(
                    out=v_sb[:, st, :, 0:HD],
                    in_=pkv[:, KVH * HD :].rearrange("p (g d) -> p g d", d=HD),
                    func=mybir.ActivationFunctionType.Copy,
                )
                norm_stats(pkv[:, 0 : KVH * HD], KVH, ssq, j * KVH)
                pkvs.append(pkv)
            inv = norm_finish(ssq, 2 * KVH, gain=False)
            for j in range(2):
                st = 2 * sp + j
                rope_apply(
                    pkvs[j][:, 0 : KVH * HD], KVH, st, k_rope[:, st, :],
                    inv, j * KVH,
                )

        # ---- Q projection over 4 q s-tiles (kv rows 256..768), halves
        # share one batched rsqrt ----
        for st in range(4):
            pqs = []
            ssq = tmp.tile([128, 16], F32, tag="ssq")
            for half in range(2):
                pq = pb.tile([128, 512], F32, tag="pb")
                for kt_ in range(8):
                    nc.tensor.matmul(
                        out=pq,
                        lhsT=xt_sb[kt_][:, 256 + st * 128 : 384 + st * 128],
                        rhs=wq_sb[kt_][:, half * 512 : half * 512 + 512],
                        start=(kt_ == 0),
                        stop=(kt_ == 7),
                    )
                norm_stats(pq, 8, ssq, half * 8)
                pqs.append(pq)
            inv = norm_finish(ssq, 16, gain=True)
            for half in range(2):
                rope_apply(
                    pqs[half], 8, st + 2,
                    q_rope[:, st, half * 512 : half * 512 + 512],
                    inv, half * 8,
                )

        # ---- transpose K: k_rope [128s, (g,d)] -> kt2_sb [2*64d, gpair, s] ----
        for st in range(6):
            ptk = pbt.tile([128, 512], BF16, tag="pbt")
            for gp in range(2):
                nc.tensor.transpose(
                    out=ptk[:, gp * 128 : gp * 128 + 128],
                    in_=k_rope[:, st, gp * 128 : gp * 128 + 128],
                    identity=ident,
                )
            nc.vector.tensor_copy(
                out=kt2_sb[:, :, st * 128 : st * 128 + 128],
                in_=ptk[:, 0:256].rearrange("p (g s) -> p g s", s=128),
            )

        # ---- transpose Q (just-in-time, interleaved with attention):
        # q_rope -> qt2_sb[:, gp*4+qb, :] (2 groups stacked) ----
        def emit_qtr(gp, qb):
            ptq = pbt.tile([128, 512], BF16, tag="pbt")
            for gl in range(2):
                g = gp * 2 + gl
                for hh in range(4):
                    h = g * 4 + hh
                    nc.tensor.transpose(
                        out=ptq[gl * 64 : gl * 64 + 64, hh * 128 : hh * 128 + 128],
                        in_=q_rope[:, qb, h * HD : h * HD + HD],
                        identity=ident,
                    )
            if (gp * 4 + qb) % 2 == 0:
                nc.scalar.activation(
                    out=qt2_sb[:, gp * 4 + qb, :], in_=ptq,
                    func=mybir.ActivationFunctionType.Copy,
                )
            else:
                nc.vector.tensor_copy(out=qt2_sb[:, gp * 4 + qb, :], in_=ptq)

        # ---- attention + O-proj, software-pipelined over it = qb*4 + g ----
        Exp = mybir.ActivationFunctionType.Exp

        def emit_scores(it):
            qb, g = it // 4, it % 4
            gp, gl = g // 2, g % 2
            att = att_pool.tile([128, 3, 512], BF16, tag="att")
            for t in range(3):
                pss = pb.tile([128, 512], F32, tag="pb")
                nc.tensor.matmul(
                    out=pss,
                    lhsT=kt2_sb[
                        gl * 64 : gl * 64 + 64, gp,
                        qb * 128 + t * 128 : qb * 128 + t * 128 + 128,
                    ],
                    rhs=qt2_sb[gl * 64 : gl * 64 + 64, gp * 4 + qb, :],
                    start=True,
                    stop=True,
                )
                kt_idx = qb + t
                if kt_idx < 2:
                    nc.scalar.activation(
                        out=att[:, t, :], in_=pss, func=Exp,
                        bias=kb_sb[:, kt_idx : kt_idx + 1],
                    )
                else:
                    nc.scalar.activation(out=att[:, t, :], in_=pss, func=Exp)
            # sliding-window masks: tile0 keep k>q, tile2 keep q>=k
            nc.gpsimd.affine_select(
                out=att[:, 0, :], in_=att[:, 0, :],
                compare_op=mybir.AluOpType.is_ge, fill=0.0,
                base=-1, channel_multiplier=1, pattern=[[0, 4], [-1, 128]],
            )
            nc.gpsimd.affine_select(
                out=att[:, 2, :], in_=att[:, 2, :],
                compare_op=mybir.AluOpType.is_ge, fill=0.0,
                base=0, channel_multiplier=-1, pattern=[[0, 4], [1, 128]],
            )
            return att

        def emit_av(it, att):
            qb, g = it // 4, it % 4
            psy = pb.tile([128, 512], F32, tag="pb")
            for t in range(3):
                nc.tensor.matmul(
                    out=psy[0:65, :],
                    lhsT=v_sb[:, qb + t, g, :],
                    rhs=att[:, t, :],
                    start=(t == 0),
                    stop=(t == 2),
                )
            # y rows to SBUF (frees the bank); fast per-it reciprocal of the
            # denominator row, broadcast via a DRAM stride-0 read
            yr = yr_pool.tile([64, 512], F32, tag="yr")
            if it % 2 == 0:
                nc.vector.tensor_copy(out=yr, in_=psy[0:64, :])
            else:
                nc.scalar.activation(
                    out=yr, in_=psy[0:64, :],
                    func=mybir.ActivationFunctionType.Copy,
                )
            dr = tn_pool.tile([1, 512], F32, tag="dr")
            nc.scalar.activation(
                out=dr, in_=psy[64:65, :],
                func=mybir.ActivationFunctionType.Copy,
            )
            rec = rb_pool.tile([1, 512], F32, tag="rec")
            nc.vector.reciprocal_approx_fast(out=rec, in_=dr)
            nc.sync.dma_start(out=recs[it : it + 1, :], in_=rec)
            rb = rb_pool.tile([64, 512], F32, tag="rb")
            nc.sync.dma_start(
                out=rb, in_=recs[it : it + 1, :].to_broadcast((64, 512))
            )
            tn = tn_pool.tile([64, 512], BF16, tag="tn")
            nc.vector.tensor_mul(out=tn, in0=yr, in1=rb)
            # scatter into yt pair layout: hh 0,2 -> partitions 0:64 of
            # pairs (2g, 2g+1); hh 1,3 -> partitions 64:128
            tn4 = tn.rearrange("p (a q) -> p a q", q=128)
            for lohalf in range(2):
                (nc.sync if lohalf == 0 else nc.gpsimd).dma_start(
                    out=yt_sb[
                        lohalf * 64 : lohalf * 64 + 64,
                        2 * g : 2 * g + 2,
                        qb * 128 : qb * 128 + 128,
                    ],
                    in_=tn4[:, lohalf::2, :],
                )

        def emit_oproj(qb):
            ob = ob_pool.tile([128, DIM], F32, tag="ob")
            for half in range(2):
                po = pb.tile([128, 512], F32, tag="pb")
                for p in range(8):
                    nc.tensor.matmul(
                        out=po,
                        lhsT=yt_sb[:, p, qb * 128 : qb * 128 + 128],
                        rhs=wo_sb[p][:, half * 512 : half * 512 + 512],
                        start=(p == 0),
                        stop=(p == 7),
                    )
                if half == 0:
                    nc.scalar.activation(
                        out=ob[:, 0:512], in_=po,
                        func=mybir.ActivationFunctionType.Copy,
                    )
                else:
                    nc.vector.tensor_copy(out=ob[:, 512:1024], in_=po)
                nc.sync.dma_start(
                    out=out[
                        qb * 128 : qb * 128 + 128, half * 512 : half * 512 + 512
                    ],
                    in_=ob[:, half * 512 : half * 512 + 512],
                )

        atts = {}
        emit_qtr(0, 0)
        emit_qtr(1, 0)
        for it in range(16):
            qb, g = it // 4, it % 4
            if g == 0 and qb < 3:
                emit_qtr(0, qb + 1)
            atts[it] = emit_scores(it)
            if g == 2 and qb < 3:
                emit_qtr(1, qb + 1)
            if it >= 1:
                emit_av(it - 1, atts.pop(it - 1))
            if it in (6, 10, 14):
                emit_oproj((it - 6) // 4)
        emit_av(15, atts.pop(15))
        emit_oproj(3)

    nc.finalize()
    return nc


def _host_inputs(x, Wq, Wk, Wv, Wo, q_gain, pair_mix):
    """Build the 8 per-core input maps."""
    x = np.asarray(x, np.float32)
    Wq = np.asarray(Wq, np.float32)
    Wk = np.asarray(Wk, np.float32)
    Wv = np.asarray(Wv, np.float32)
    Wo = np.asarray(Wo, np.float32)
    q_gain = np.asarray(q_gain, np.float32)
    pair_mix = np.asarray(pair_mix, np.float32)

    # fold pair mixing into Wo:  out = y_mix @ Wo.T,  y_mix = y @ M.T  =>  Wo' = Wo @ M
    M = np.zeros((DIM, DIM), np.float32)
    eye = np.eye(HD, dtype=np.float32)
    for p in range(H // 2):
        for o in range(2):
            for i in range(2):
                ho, hi = 2 * p + o, 2 * p + i
                M[ho * HD : ho * HD + HD, hi * HD : hi * HD + HD] = (
                    pair_mix[p, o, i] * eye
                )
    woT = np.ascontiguousarray((Wo @ M).T)

    wqT = np.ascontiguousarray(Wq.T)
    wkvT = np.ascontiguousarray(
        np.concatenate([Wk.T, Wv.T], axis=1)
    )  # [DIM, 512]
    qg8 = (q_gain / math.sqrt(HD)).reshape(1, H).astype(np.float32)

    inv_freq = 1.0 / (ROPE_BASE ** (np.arange(0, HD, 2, dtype=np.float32) / HD))

    import ml_dtypes
    bf = ml_dtypes.bfloat16
    wqT, wkvT, woT = (a.astype(bf) for a in (wqT, wkvT, woT))
    in_maps = []
    for core in range(NCORES):
        b, c = core // 4, core % 4
        ks = 512 * c - 256
        xc = np.zeros((NK, DIM), np.float32)
        lo = max(0, ks)
        xc[lo - ks :] = x[b, lo : ks + NK]
        t = (ks + np.arange(NK, dtype=np.float32))[:, None]
        freqs = t * inv_freq[None, :]
        # exp-bias columns for k-tiles 0 and 1 (the halo region): -30000
        # where the global position is < 0 (chunk 0 only), else 0
        kbcols = np.zeros((128, 2), np.float32)
        for tt in range(2):
            gpos = ks + tt * 128 + np.arange(128)
            kbcols[:, tt] = np.where(gpos < 0, -30000.0, 0.0)
        in_maps.append(
            {
                "xt": np.ascontiguousarray(xc.T).astype(bf),
                "wq": wqT,
                "wkv": wkvT,
                "wo": woT,
                "cosk": np.cos(freqs).astype(bf),
                "sink": np.sin(freqs).astype(bf),
                "kb2": kbcols,
                "qgain": qg8,
            }
        )
    return in_maps


def kernel(x, Wq, Wk, Wv, Wo, q_gain, pair_mix):
    global _BUILT
    from concourse.bass_utils import run_bass_kernel_spmd

    if _BUILT is None:
        _BUILT = _build()
    in_maps = _host_inputs(x, Wq, Wk, Wv, Wo, q_gain, pair_mix)
    res = run_bass_kernel_spmd(_BUILT, in_maps, list(range(NCORES)))
    out = np.empty((B, S, DIM), np.float32)
    for core in range(NCORES):
        b, c = core // 4, core % 4
        out[b, 512 * c : 512 * c + 512, :] = res.results[core]["out"]
    return out
